# revision 21
# baseline (speedup 1.0000x reference)
"""Trainium2 Bass kernel for nn_AttentionOutput (complex causal leaky-relu attention).

Reference (B=4, N=4096, F=64), per batch:
    sr = (Qr@Kr^T - Qi@Ki^T)/sqrt(N); si = (Qr@Ki^T + Qi@Kr^T)/sqrt(N)
    wr = tril * leaky_relu(sr);        wi = tril * leaky_relu(si)
    out_r = (wr@Vr)@W_att^T + b;       out_i = (wi@Vi)@W_att^T + b

Distribution: 2 cores per batch.  Core parity h processes j-blocks J === h
(mod 2) for ALL 4096 query rows; causal work is then identical across cores
(slot I needs 2I+2 j-blocks), so a single SPMD program serves all 8 cores and
the host sums the two partial outputs per batch.

Host-side layout prep removes every on-device transpose:
  - scores contract over p = f*2+c (128 partitions, ONE matmul per component):
    sr = Qmodr . K^T where Qmodr = Q with odd columns negated, and
    si = Qmodi . K^T where Qmodi = Q with column pairs swapped; K stays plain.
    Both Q variants are fed pre-transposed [128, N].
  - V' = (1/64) V @ W_att^T folds the score scale and the output projection
    into the attention-value matmul (leaky_relu is positively homogeneous).
  - output is stored transposed, bf16 ([128, N]: y_r^T rows 0:64, y_i^T rows
    64:128); the host untransposes, adds bias + correction, sums parities.

leaky_relu lowering: leaky(s) = 0.99*relu(s) + 0.01*s.  For causally-full
j-blocks the 0.01*s term telescopes into a per-slot constant matmul which is
computed ON THE HOST in fp32 (vs the relu'd partials from the device) and
added during gather -- no correction matmuls on the PE.  Diagonal tiles
compute u = (mask/0.99)*s on DVE (drains PSUM), w = Lrelu(u) on ACT (SBUF
source only: ACT Lrelu reading PSUM hangs TRN2 -- never emit that), and one
value matmul against 0.99*V' reconstructs V'*leaky exactly (homogeneity).

Pipeline (PE-throughput oriented, ~83us/core on HW):
  - flat (slot, j-block) sequence; score matmuls run TWO blocks ahead of the
    value matmuls so PSUM drains stay off the PE critical path.  6 score
    PSUM banks + 2 packed y banks = all 8 banks.
  - y_r/y_i packed into ONE PSUM bank (partitions 0:64 / 64:128) via PE
    column tiling (out.base_partition=64 for the i half).
  - full-tile relu drains alternate ACT (Relu activation) / DVE (max).
  - the 2nd diagonal block per slot is identically zero in columns 0:256
    for both parities: its drain chain and value matmul cover only the live
    half; the 1st (full-width) diag matmul is emitted last with stop=True.
  - slots processed 1..7 then 0, so the kernel tail is slot 0 (2 blocks)
    and a 128KB output split across both DMA queues; big outputs overlap
    compute.  Input DMAs are issued in first-use order across the two HW
    DGE queues (Sync + Activation).
  - y-bank copies deferred ~2 blocks so they never delay next-slot drains.
"""

import numpy as np

import concourse.bacc as bacc
import concourse.tile as tile
from concourse import mybir
from concourse.bass_utils import run_bass_kernel_spmd

B, N, F = 4, 4096, 64
P = 128             # = 2*F: score contraction width / partition count
JB = 128            # j-block width
IBW = 512           # i-block (slot) width
NSLOT = N // IBW    # 8 slots
NJPAR = N // JB // 2  # 16 parity j-blocks per core
NEG = 0.01
SCALE = 1.0 / 64.0  # 1/sqrt(N)
NCORES = 8

_DT = mybir.dt.float32
MM_BF16 = True      # bf16 matmul inputs: full PE stream rate, half the DMA
SKIP_LDW = True     # s_i reuses the kp stationary loaded by s_r
_CACHE: dict = {}


def _build_nc():
    nc = bacc.Bacc("TRN2", target_bir_lowering=False, num_devices=NCORES)
    dt = _DT
    mdt = mybir.dt.bfloat16 if MM_BF16 else _DT  # matmul input dtype
    qrT = nc.dram_tensor("qrT", [P, N], mdt, kind="ExternalInput")
    qiT = nc.dram_tensor("qiT", [P, N], mdt, kind="ExternalInput")
    kp = nc.dram_tensor("kp", [P, NJPAR * JB], mdt, kind="ExternalInput")
    # va = 0.99 * V' (relu term), vb = 0.01 * V' (raw term, diagonal only)
    var_ = nc.dram_tensor("var", [P, NJPAR * F], mdt, kind="ExternalInput")
    vai = nc.dram_tensor("vai", [P, NJPAR * F], mdt, kind="ExternalInput")
    dmask = nc.dram_tensor("dmask", [2, JB, IBW], mdt, kind="ExternalInput")
    out = nc.dram_tensor("out", [P, N], mdt, kind="ExternalOutput")

    relu = mybir.ActivationFunctionType.Relu
    lrelu = mybir.ActivationFunctionType.Lrelu
    mul_op = mybir.AluOpType.mult

    with tile.TileContext(nc) as tc:
        with (
            tc.tile_pool(name="res", bufs=1) as res,
            tc.tile_pool(name="wp", bufs=20) as wp,
            tc.tile_pool(name="osb", bufs=2) as osb,
            tc.tile_pool(name="spsum", bufs=6, space="PSUM") as spsum,
            tc.tile_pool(name="ypsum", bufs=2, space="PSUM") as ypsum,
        ):
            sb_qr = res.tile([P, N], mdt, tag="qr")
            sb_qi = res.tile([P, N], mdt, tag="qi")
            sb_k = res.tile([P, NJPAR * JB], mdt, tag="k")
            sb_var = res.tile([P, NJPAR * F], mdt, tag="var")
            sb_vai = res.tile([P, NJPAR * F], mdt, tag="vai")
            sb_m0 = res.tile([JB, IBW], mdt, tag="m0")
            sb_m1 = res.tile([JB, IBW], mdt, tag="m1")

            def dma(dst, src, c):
                sl = slice(c * 512, (c + 1) * 512)
                nc.sync.dma_start(out=dst[:, sl], in_=src[:, sl])

            # DMAs ordered by first-use time (processing order 1..7,0),
            # split across the two HW DGE queues (Sync + Activation).
            nc.sync.dma_start(out=sb_qr[:, 512:768], in_=qrT[:, 512:768])
            nc.scalar.dma_start(out=sb_qr[:, 768:1024], in_=qrT[:, 768:1024])
            nc.sync.dma_start(out=sb_k[:, 0:512], in_=kp[:, 0:512])
            nc.scalar.dma_start(out=sb_qi[:, 512:1024], in_=qiT[:, 512:1024])
            nc.scalar.dma_start(out=sb_m0, in_=dmask[0])
            nc.scalar.dma_start(out=sb_m1, in_=dmask[1])
            dma(sb_var, var_, 0)
            dma(sb_qr, qrT, 2)
            nc.scalar.dma_start(out=sb_vai[:, 0:512], in_=vai[:, 0:512])
            nc.scalar.dma_start(out=sb_qi[:, 1024:1536], in_=qiT[:, 1024:1536])
            dma(sb_k, kp, 1)
            dma(sb_qr, qrT, 3)
            dma(sb_qi, qiT, 3)
            dma(sb_qr, qrT, 4)
            dma(sb_qi, qiT, 4)
            dma(sb_k, kp, 2)
            dma(sb_var, var_, 1)
            dma(sb_vai, vai, 1)
            dma(sb_qr, qrT, 5)
            dma(sb_qi, qiT, 5)
            dma(sb_k, kp, 3)
            nc.sync.dma_start(out=sb_qr[:, 3072:4096], in_=qrT[:, 3072:4096])
            nc.sync.dma_start(out=sb_qi[:, 3072:4096], in_=qiT[:, 3072:4096])
            nc.sync.dma_start(out=sb_qr[:, 0:512], in_=qrT[:, 0:512])
            nc.sync.dma_start(out=sb_qi[:, 0:512], in_=qiT[:, 0:512])

            sb_masks = (sb_m0, sb_m1)
            # slot 0 (2 blocks) processed LAST: the final output transfer is
            # only 128KB (split over both DMA queues) and slot 7's large
            # output overlaps slot-0 compute.
            ORDER = [1, 2, 3, 4, 5, 6, 7, 0]
            seq = [(s, p) for s in ORDER for p in range(2 * s + 2)]
            pend = {}    # idx -> per-comp drained tiles
            ytile = {}   # slot -> packed PSUM bank [P, IBW]
            drain_ctr = 0
            copyq = []   # (position, slot, y psum tile) awaiting copy-out

            def flush_copies(done_pos):
                while copyq and copyq[0][0] <= done_pos:
                    pos3, s3, y3 = copyq.pop(0)
                    y_sb = osb.tile([P, IBW], mdt, tag="ysb",
                                    name=f"ysb{s3}")
                    nc.scalar.copy(y_sb[:], y3[:])
                    osl = slice(s3 * IBW, (s3 + 1) * IBW)
                    if s3 == 0:
                        nc.sync.dma_start(out=out[:, 0:IBW // 2],
                                          in_=y_sb[:, 0:IBW // 2])
                        nc.scalar.dma_start(out=out[:, IBW // 2:IBW],
                                            in_=y_sb[:, IBW // 2:IBW])
                    elif pos3 % 2 == 0:
                        nc.sync.dma_start(out=out[:, osl], in_=y_sb[:])
                    else:
                        nc.scalar.dma_start(out=out[:, osl], in_=y_sb[:])

            for idx in range(len(seq) + 2):
                if idx < len(seq):
                    s, p = seq[idx]
                    cnt = 2 * s + 2
                    isl = slice(s * IBW, (s + 1) * IBW)
                    if p == 0:
                        y = ytile[s] = ypsum.tile([P, IBW], dt, tag="y",
                                                  name=f"y{s}")
                    # scores: s_i reuses the kp stationary loaded by s_r
                    ksl = slice(p * JB, (p + 1) * JB)
                    s_r = spsum.tile([JB, IBW], dt, tag="s")
                    nc.tensor.matmul(s_r[:], sb_k[:, ksl], sb_qr[:, isl],
                                     start=True, stop=True)
                    s_i = spsum.tile([JB, IBW], dt, tag="s")
                    mm_i = nc.tensor.matmul(s_i[:], sb_k[:, ksl],
                                            sb_qi[:, isl],
                                            start=True, stop=True)
                    if SKIP_LDW:
                        mm_i.ins.ldweights = False
                    # drains (off the PE critical path; values lag 2 blocks)
                    tiles = []
                    for s_ps in (s_r, s_i):
                        if p < cnt - 2:
                            w = wp.tile([JB, IBW], mdt, tag="w")
                            if drain_ctr % 2 == 1:
                                nc.vector.tensor_scalar_max(w[:], s_ps[:], 0.0)
                            else:
                                nc.scalar.activation(w[:], s_ps[:], relu)
                            drain_ctr += 1
                            tiles.append(w)
                        else:
                            # mask pre-scaled by 1/0.99; Lrelu(u)*0.99V' ==
                            # V'*leaky(mask*s) by positive homogeneity.
                            # 2nd diag block (pp=1) is all-zero in columns
                            # 0:256 for both parities: drain the live half.
                            pp = p - (cnt - 2)
                            lo = 256 if (pp == 1 and s > 0) else 0
                            mk = sb_masks[pp]
                            u = wp.tile([JB, IBW - lo], mdt, tag="u")
                            nc.vector.tensor_tensor(out=u[:],
                                                    in0=s_ps[:, lo:IBW],
                                                    in1=mk[:, lo:IBW],
                                                    op=mul_op)
                            w = wp.tile([JB, IBW - lo], mdt, tag="w")
                            nc.scalar.activation(w[:], u[:], lrelu)
                            tiles.append(w)
                    pend[idx] = tiles
                if idx >= 2:
                    s2, p2 = seq[idx - 2]
                    pos2 = ORDER.index(s2)
                    if p2 >= 2 or pos2 == len(ORDER) - 1:
                        flush_copies(pos2 - 1)
                    cnt2 = 2 * s2 + 2
                    y = ytile[s2]
                    vsl = slice(p2 * F, (p2 + 1) * F)
                    tiles = pend.pop(idx - 2)
                    if p2 == cnt2 - 2 and s2 > 0:
                        # defer the 1st diag block so the full-width matmul
                        # is emitted last and carries stop=True for the
                        # whole accumulation group
                        pend[('d0', s2)] = tiles
                    elif p2 == cnt2 - 1 and s2 > 0:
                        vsl0 = slice((p2 - 1) * F, p2 * F)
                        t0 = pend.pop(('d0', s2))
                        for comp, (sb_va, psl) in enumerate((
                                (sb_var, slice(0, 64)),
                                (sb_vai, slice(64, 128)))):
                            # live half of the 2nd diag block
                            nc.tensor.matmul(y[psl, 256:IBW], sb_va[:, vsl],
                                             tiles[comp][:],
                                             start=False, stop=False)
                            # full-width 1st diag block closes the group
                            nc.tensor.matmul(y[psl, :], sb_va[:, vsl0],
                                             t0[comp][:],
                                             start=False, stop=True)
                    else:
                        for comp, (sb_va, psl) in enumerate((
                                (sb_var, slice(0, 64)),
                                (sb_vai, slice(64, 128)))):
                            first = (p2 == 0)
                            last = (p2 == cnt2 - 1)
                            nc.tensor.matmul(y[psl, :], sb_va[:, vsl],
                                             tiles[comp][:],
                                             start=first, stop=last)
                    if p2 == cnt2 - 1:
                        # copy emission deferred ~2 blocks so it never
                        # delays the next slot's drains on ACT.
                        copyq.append((pos2, s2, y))
            flush_copies(len(ORDER) - 1)
    nc.compile()
    return nc


def _prep_inputs(Q, K, V, W_att, b_att):
    """Host-side re-layout: per-core in_maps for run_bass_kernel_spmd."""
    Q = np.asarray(Q, dtype=np.float32)
    K = np.asarray(K, dtype=np.float32)
    V = np.asarray(V, dtype=np.float32)
    W_att = np.asarray(W_att, dtype=np.float32)

    Qf = Q.reshape(B, N, P)          # [b, i, f*2+c]
    Kf = K.reshape(B, N, P)
    Vpr = SCALE * (V[..., 0] @ W_att.T)   # [B, N, F]
    Vpi = SCALE * (V[..., 1] @ W_att.T)

    # causal masks for a slot's last two parity j-blocks, per core parity h:
    # diagonal sub-block d = 2k+h of the slot's group of 4
    jj = np.arange(JB)[:, None]
    ii = np.arange(IBW)[None, :]
    mscale = 1.0 / (1.0 - NEG)
    masks = {h: np.stack([mscale * (ii >= jj + JB * (2 * k + h))
                          .astype(np.float32) for k in range(2)])
             for h in (0, 1)}

    if MM_BF16:
        import ml_dtypes
        cvt = lambda a: np.ascontiguousarray(a).astype(ml_dtypes.bfloat16)
    else:
        cvt = lambda a: np.ascontiguousarray(a, dtype=np.float32)

    in_maps = []
    corrs = []
    for c in range(NCORES):
        b, h = divmod(c, 2)
        Qmodr = Qf[b].copy()
        Qmodr[:, 1::2] *= -1.0
        Qmodi = np.empty_like(Qf[b])
        Qmodi[:, 0::2] = Qf[b][:, 1::2]
        Qmodi[:, 1::2] = Qf[b][:, 0::2]
        # parity-packed K: [P, NJPAR*JB], position pp holds block J = 2*pp+h
        kp3 = Kf[b].reshape(N // JB, JB, P)[h::2]          # [16, j, p]
        kp = kp3.transpose(2, 0, 1).reshape(P, -1)         # [p, pp*JB+j]
        vr3 = Vpr[b].reshape(N // JB, JB, F)[h::2]         # [16, j, f]
        vi3 = Vpi[b].reshape(N // JB, JB, F)[h::2]
        vpr = vr3.transpose(1, 0, 2).reshape(JB, -1)       # [j, pp*F+f]
        vpi = vi3.transpose(1, 0, 2).reshape(JB, -1)
        # per-slot correction: 0.01 * sum over FULL blocks (pos < cnt-2 = 2s)
        prod_r = np.einsum('bjp,bjf->bpf', kp3, vr3)       # [16, p, f]
        prod_i = np.einsum('bjp,bjf->bpf', kp3, vi3)
        pre_r = np.concatenate(
            [np.zeros((1, P, F), np.float32), np.cumsum(prod_r, axis=0)])
        pre_i = np.concatenate(
            [np.zeros((1, P, F), np.float32), np.cumsum(prod_i, axis=0)])
        # host-side correction: corr[i, f] per slot = 0.01 * q_slot^T pre[2s]
        corr = np.empty((N, F, 2), dtype=np.float32)
        for s in range(NSLOT):
            isl = slice(s * IBW, (s + 1) * IBW)
            corr[isl, :, 0] = NEG * (Qmodr[isl] @ pre_r[2 * s])
            corr[isl, :, 1] = NEG * (Qmodi[isl] @ pre_i[2 * s])
        corrs.append(corr)
        in_maps.append({
            "qrT": cvt(Qmodr.T),
            "qiT": cvt(Qmodi.T),
            "kp": cvt(kp),
            "var": cvt((1.0 - NEG) * vpr),
            "vai": cvt((1.0 - NEG) * vpi),
            "dmask": cvt(masks[h]),
        })
    return in_maps, corrs


def _gather(results, corrs, b_att):
    b_att = np.asarray(b_att, dtype=np.float32)
    out = np.empty((B, N, F, 2), dtype=np.float32)
    for b in range(B):
        y = (results[2 * b]["out"].astype(np.float32)
             + results[2 * b + 1]["out"].astype(np.float32))  # [128, N]
        corr = corrs[2 * b] + corrs[2 * b + 1]
        out[b, :, :, 0] = y[0:64].T + corr[:, :, 0] + b_att[None, :]
        out[b, :, :, 1] = y[64:128].T + corr[:, :, 1] + b_att[None, :]
    return out


def kernel(Q, K, V, W_att, b_att):
    if "nc" not in _CACHE:
        _CACHE["nc"] = _build_nc()
    nc = _CACHE["nc"]
    in_maps, corrs = _prep_inputs(Q, K, V, W_att, b_att)
    res = run_bass_kernel_spmd(nc, in_maps, core_ids=list(range(NCORES)))
    return _gather(res.results, corrs, b_att)
